# revision 41
# baseline (speedup 1.0000x reference)
"""Trainium2 Bass kernel for nn_MixtureOfTokensLayer.

Math (per sequence position s; B=32 tokens form ONE group of k=32):
  logits = x_s @ controller            (32, 8)
  w      = softmax_k(logits)           (32, 8)
  merged = w.T @ x_s                   (8, 1024)   per-expert token mix
  h      = relu(merged_e @ lin1_e)     (8, 512)
  eo     = h_e @ lin2_e                (8, 1024)
  out_s  = w @ eo                      (32, 1024)

Sharding: data-parallel over S (2048 -> 256 per core, 8 cores). No
collectives. All compute in bf16 with fp32 PSUM accumulation; output
returned to host as bf16 and upcast (error stays ~6e-3 rel).

Per-core dataflow (block = 4 s-positions = 128 tokens on partitions),
pipelined over 2 s-chunks of 128 s; chunk B's P1 (input DMA + routing) is
source-interleaved with chunk A's P2b/P3 (MLP tail + emit + output DMA)
so input and output DMA overlap:
  P1 per 16-s tile: logits^T = ctl.T @ xT (PE) -> PE-transpose 128-token
     chunks -> exp (ACT, no max-subtraction: logits are O(5)) -> build
     block-diagonal wbd (128 tok x 32 (s,e)) with one masked multiply ->
     merge MM lhsT=wbd gives merged with (s,e) ON PARTITIONS; Z = wbd.T @
     ones (same lhsT); 1/Z folded into the PSUM evacuations (relu
     positive-homogeneity lets normalization commute); PE-transpose
     merged -> merged^T (d-major); PE-transpose wbd -> wbdT (emit lhsT).
  P2a: lin1 per expert (w1 streamed per chunk, ACT DGE ring).
  P2b + P3 per d-half: lin2 (w2 resident); PE-transpose eo^T ->
     (s,e)-row blocks; emit MM out = wbdT_scaled.T @ eo_block; evac bf16
     (DVE/ACT alternating); DMA out on the ACT DGE ring.
"""

import os
import sys

import numpy as np
import ml_dtypes

sys.path.insert(0, "/opt/trn_rl_repo")

import concourse.bass as bass
import concourse.mybir as mybir
import concourse.tile as tile
from concourse import bacc

B, S, D, F, E, K = 32, 2048, 1024, 512, 8, 32
N_CORES = 8
TILE_S = 16              # s-positions per P1/P3 tile
NBLK = 4                 # blocks per tile
TOK = 128                # tokens per block (4 s * 32 k)
DC = D // 128            # 8 d-chunks
FC = F // 128            # 4 f-chunks
CHUNK_S = 128            # s-positions per pipeline chunk
BF = mybir.dt.bfloat16
F32 = mybir.dt.float32
AF = mybir.ActivationFunctionType


class _Ctx:
    pass


def moe_body(tc, xg, xT, w1, w2, ctl, idf, idb, msk, out, s_local, reps=1):
    nc = tc.nc
    import contextlib

    with (
        tc.tile_pool(name="const", bufs=1) as const,
        tc.tile_pool(name="resw", bufs=1) as resw,
        tc.tile_pool(name="persist", bufs=2) as persist,
        tc.tile_pool(name="pers1", bufs=1) as pers1,
        tc.tile_pool(name="xstream", bufs=2) as xsp,
        tc.tile_pool(name="wstream", bufs=2) as wsp,
        tc.tile_pool(name="p1", bufs=2) as p1,
        tc.tile_pool(name="outst", bufs=4) as outst,
        tc.tile_pool(name="ps_bigA", bufs=2, space="PSUM") as ps_big,
        tc.tile_pool(name="ps_bigB", bufs=2, space="PSUM") as ps_out,
        tc.tile_pool(name="ps_sm", bufs=4, space="PSUM") as ps_sm,
    ):
        c = _Ctx()
        c.tc, c.nc = tc, nc
        c.xsp, c.wsp, c.p1, c.persist, c.pers1, c.outst = (
            xsp, wsp, p1, persist, pers1, outst)
        c.ps_big, c.ps_out, c.ps_sm = ps_big, ps_out, ps_sm
        c.w1, c.w2 = w1, w2

        # ---- constants ----
        c.ctl_sb = const.tile([128, DC, E], BF, name="ctl_sb")
        nc.sync.dma_start(c.ctl_sb[:], ctl)
        c.idf_sb = const.tile([128, 128], F32, name="idf_sb")
        nc.sync.dma_start(c.idf_sb[:], idf)
        c.idb_sb = const.tile([128, 128], BF, name="idb_sb")
        nc.sync.dma_start(c.idb_sb[:], idb)
        c.msk_sb = const.tile([128, NBLK, E], BF, name="msk_sb")
        nc.sync.dma_start(c.msk_sb[:], msk)
        c.ones_sb = const.tile([128, 1], BF, name="ones_sb")
        nc.vector.memset(c.ones_sb[:], 1.0)
        # resident lin2 (read once; per-dt so x loads interleave)
        c.w2_sb = resw.tile([128, DC, FC, E, 128], BF, name="w2_sb")
        for dt in range(DC):
            nc.scalar.dma_start(c.w2_sb[:, dt], w2[:, dt])

        c.xg_r = xg.rearrange("(nb p) d -> p nb d", p=128)
        c.xT_r = xT.rearrange("(dc p) t -> p dc t", p=128)
        c.out_r = out.rearrange("(nb p) d -> p nb d", p=128)

        n_chunks = max(1, s_local // CHUNK_S)
        chunk_s = s_local // n_chunks
        rep_ctx = tc.For_i(0, reps, 1) if reps > 1 else contextlib.nullcontext()
        with rep_ctx:
            prev_p23 = None
            for chunk in range(n_chunks):
                st = _chunk_state(c, chunk, chunk_s)
                gen1 = _p1_steps(c, st)
                if prev_p23 is None:
                    _drain(gen1)
                else:
                    _interleave(prev_p23, gen1)
                _drain(_p2a_steps(c, st))
                prev_p23 = _p23_steps(c, st)
            if prev_p23 is not None:
                _drain(prev_p23)


def _drain(g):
    for _ in g:
        pass


def _interleave(g_slow, g_fast):
    """Alternate emitting ~3 steps of g_slow per step of g_fast, then drain."""
    done_s = done_f = False
    while not (done_s and done_f):
        for _ in range(3):
            if not done_s:
                done_s = next(g_slow, _SENTINEL) is _SENTINEL
        if not done_f:
            done_f = next(g_fast, _SENTINEL) is _SENTINEL


_SENTINEL = object()


def _chunk_state(c, chunk, chunk_s):
    st = _Ctx()
    st.chunk = chunk
    st.cs = chunk_s
    st.n_tiles = chunk_s // TILE_S
    st.tile0 = chunk * st.n_tiles
    st.mT = c.persist.tile([128, DC, chunk_s, E], BF, tag="mT",
                           name=f"mT{chunk}")
    st.wbdT = c.persist.tile([128, st.n_tiles, TOK], BF, tag="wbdT",
                             name=f"wbdT{chunk}")
    st.h_all = c.pers1.tile([128, FC, E, chunk_s], BF, tag="h",
                            name=f"h{chunk}")
    st.eoT = c.pers1.tile([128, DC, chunk_s, E], BF, tag="eoT",
                          name=f"eoT{chunk}")
    return st


def _p1_steps(c, st):
    nc = c.nc
    for t in range(st.n_tiles):
        ti = st.tile0 + t
        xT_t = c.xsp.tile([128, DC, TILE_S * K], BF, tag="xT", name=f"xT{ti}")
        nc.sync.dma_start(xT_t[:], c.xT_r[:, :, ti * 512:(ti + 1) * 512])
        xg_t = c.xsp.tile([128, NBLK, D], BF, tag="xg", name=f"xg{ti}")
        nc.sync.dma_start(xg_t[:], c.xg_r[:, ti * NBLK:(ti + 1) * NBLK, :])

        # logits^T (e, 512 tokens), accumulate over d-chunks
        lg_ps = c.ps_sm.tile([8, 512], F32, tag="sm", name=f"lgps{ti}")
        for dc in range(DC):
            nc.tensor.matmul(lg_ps[:], c.ctl_sb[:, dc, :], xT_t[:, dc, :],
                             start=(dc == 0), stop=(dc == DC - 1))
        lgT = c.p1.tile([8, 512], F32, tag="lgT", name=f"lgT{ti}")
        nc.vector.tensor_copy(out=lgT[:], in_=lg_ps[:])

        # per block: transpose 128-token chunk, exp
        expl = c.p1.tile([128, NBLK, E], BF, tag="expl", name=f"expl{ti}")
        for b in range(NBLK):
            tp_ps = c.ps_sm.tile([128, 8], F32, tag="sm", name=f"tpps{ti}_{b}")
            nc.tensor.transpose(tp_ps[:], lgT[:, b * TOK:(b + 1) * TOK],
                                c.idf_sb[:8, :8])
            nc.scalar.activation(expl[:, b, :], tp_ps[:], AF.Exp)

        mg_ps = [c.ps_big.tile([128, 512], F32, tag="big", name=f"mgps{ti}{h}")
                 for h in range(2)]
        z_ps = c.ps_sm.tile([128, 1], F32, tag="sm", name=f"zps{ti}")
        wt_ps = c.ps_sm.tile([128, TOK], BF, tag="sm", name=f"wtps{ti}")
        for b in range(NBLK):
            # block-diag wbd (128 tok, 4s x 8e) via one masked multiply
            wbd = c.p1.tile([TOK, NBLK, E], BF, tag="wbd", name=f"wbd{ti}_{b}")
            nc.vector.tensor_tensor(
                out=wbd[:],
                in0=expl[:, b, None, :].to_broadcast((TOK, NBLK, E)),
                in1=c.msk_sb[:],
                op=mybir.AluOpType.mult)
            r0 = 32 * b
            nc.tensor.matmul(z_ps[r0:r0 + 32, :], wbd[:], c.ones_sb[:],
                             start=True, stop=True, tile_position=(0, r0))
            nc.tensor.matmul(mg_ps[0][r0:r0 + 32, :], wbd[:],
                             xg_t[:, b, 0:512], start=True, stop=True,
                             tile_position=(0, r0))
            nc.tensor.matmul(mg_ps[1][r0:r0 + 32, :], wbd[:],
                             xg_t[:, b, 512:1024], start=True, stop=True,
                             tile_position=(0, r0))
            nc.tensor.transpose(wt_ps[r0:r0 + 32, :], wbd[:], c.idb_sb[:],
                                tile_position=(0, r0))

        zr = c.p1.tile([128, 1], F32, tag="zr", name=f"zr{ti}")
        nc.vector.reciprocal(zr[:], z_ps[:])
        # evacuations with 1/Z folded in (per-partition scalar = per (s,e))
        mg_sb = c.p1.tile([128, D], BF, tag="mgsb", name=f"mgsb{ti}")
        nc.vector.tensor_scalar_mul(mg_sb[:, 0:512], mg_ps[0][:], zr[:])
        nc.vector.tensor_scalar_mul(mg_sb[:, 512:1024], mg_ps[1][:], zr[:])
        nc.vector.tensor_scalar_mul(st.wbdT[:, t, :], wt_ps[:], zr[:])

        # merged -> merged^T (d on partitions); 4 transposes per evac
        for g in range(2):
            mt_ps = c.ps_sm.tile([128, 4, 128], BF, tag="sm",
                                 name=f"mtps{ti}_{g}")
            for j in range(4):
                dc = g * 4 + j
                nc.tensor.transpose(mt_ps[:, j, :],
                                    mg_sb[:, dc * 128:(dc + 1) * 128],
                                    c.idb_sb[:])
            nc.vector.tensor_copy(
                out=st.mT[:, g * 4:(g + 1) * 4,
                          t * TILE_S:(t + 1) * TILE_S, :],
                in_=mt_ps[:].rearrange("p j (s e) -> p j s e", e=E))
        yield


def _p2a_steps(c, st):
    nc = c.nc
    for e in range(E):
        w1_t = c.wsp.tile([128, DC, F], BF, tag="w1", name=f"w1_{st.chunk}_{e}")
        nc.scalar.dma_start(w1_t[:], c.w1[:, :, e, :])
        h_ps = c.ps_big.tile([128, FC, st.cs], F32, tag="big",
                             name=f"hps{st.chunk}_{e}")
        for ft in range(FC):
            for dc in range(DC):
                nc.tensor.matmul(h_ps[:, ft, :],
                                 w1_t[:, dc, ft * 128:(ft + 1) * 128],
                                 st.mT[:, dc, :, e],
                                 start=(dc == 0), stop=(dc == DC - 1))
        nc.scalar.activation(st.h_all[:, :, e, :], h_ps[:], AF.Relu)
        yield


def _p23_steps(c, st):
    nc = c.nc
    for half in range(2):
        for dt in range(half * 4, half * 4 + 4):
            for eg in range(2):           # 4-expert groups share one psum bank
                eo_ps = c.ps_big.tile([128, 4, st.cs], F32, tag="big",
                                      name=f"eops{st.chunk}_{dt}_{eg}")
                for j in range(4):
                    e = eg * 4 + j
                    for fc in range(FC):
                        nc.tensor.matmul(eo_ps[:, j, :],
                                         c.w2_sb[:, dt, fc, e, :],
                                         st.h_all[:, fc, e, :],
                                         start=(fc == 0), stop=(fc == FC - 1))
                ev = nc.scalar.copy if (dt + eg) % 2 else nc.vector.tensor_copy
                ev(out=st.eoT[:, dt, :, eg * 4:(eg + 1) * 4],
                   in_=eo_ps[:].rearrange("p e s -> p s e"))
                yield

        # emit for this d-half
        for t in range(st.n_tiles):
            ti = st.tile0 + t
            eo_blk = c.p1.tile([128, 4, 128], BF, tag="eoblk",
                               name=f"eoblk{half}_{ti}")
            et_ps = c.ps_sm.tile([128, 4, 128], BF, tag="sm",
                                 name=f"etps{half}_{ti}")
            for j in range(4):
                dt = half * 4 + j
                src = st.eoT[:, dt, t * TILE_S:(t + 1) * TILE_S, :]
                nc.tensor.transpose(et_ps[:, j, :],
                                    src.rearrange("p s e -> p (s e)"),
                                    c.idb_sb[:])
            ev = nc.scalar.copy if t % 2 else nc.vector.tensor_copy
            ev(out=eo_blk[:], in_=et_ps[:])
            for b in range(NBLK):
                o_ps = c.ps_out.tile([128, 512], F32, tag="out",
                                     name=f"ops{half}_{ti}_{b}")
                r0 = 32 * b
                nc.tensor.matmul(o_ps[:], st.wbdT[r0:r0 + 32, t, :],
                                 eo_blk[r0:r0 + 32, :, :],
                                 start=True, stop=True, tile_position=(r0, 0))
                o_sb = c.outst.tile([128, 512], BF, tag="osb",
                                    name=f"osb{half}_{ti}_{b}")
                if b % 2 == 0:
                    nc.vector.tensor_copy(out=o_sb[:], in_=o_ps[:])
                else:
                    nc.scalar.copy(out=o_sb[:], in_=o_ps[:])
                nc.scalar.dma_start(
                    c.out_r[:, ti * NBLK + b, half * 512:(half + 1) * 512],
                    o_sb[:])
            yield


def build_module(s_local, num_devices, reps=1):
    T = s_local * K
    nc = bacc.Bacc("TRN2", target_bir_lowering=False, debug=False,
                   num_devices=num_devices)
    xg = nc.dram_tensor("xg", [T, D], BF, kind="ExternalInput").ap()
    xT = nc.dram_tensor("xT", [D, T], BF, kind="ExternalInput").ap()
    w1 = nc.dram_tensor("w1", [128, DC, E, F], BF, kind="ExternalInput").ap()
    w2 = nc.dram_tensor("w2", [128, DC, FC, E, 128], BF,
                        kind="ExternalInput").ap()
    ctl = nc.dram_tensor("ctl", [128, DC, E], BF, kind="ExternalInput").ap()
    idf = nc.dram_tensor("idf", [128, 128], F32, kind="ExternalInput").ap()
    idb = nc.dram_tensor("idb", [128, 128], BF, kind="ExternalInput").ap()
    msk = nc.dram_tensor("msk", [128, NBLK, E], BF, kind="ExternalInput").ap()
    out = nc.dram_tensor("out", [T, D], BF, kind="ExternalOutput").ap()
    with tile.TileContext(nc) as tc:
        moe_body(tc, xg, xT, w1, w2, ctl, idf, idb, msk, out, s_local,
                 reps=reps)
    nc.compile()
    return nc


def stage_weights(lin1, lin2, controller):
    bf = ml_dtypes.bfloat16
    w1h = np.ascontiguousarray(
        lin1.reshape(E, DC, 128, F).transpose(2, 1, 0, 3)).astype(bf)
    # (128p, dt, fc, e, 128c): element = lin2[e, fc*128+p, dt*128+c]
    w2h = np.ascontiguousarray(
        lin2.reshape(E, FC, 128, DC, 128).transpose(2, 3, 1, 0, 4)).astype(bf)
    ctlh = np.ascontiguousarray(
        controller.reshape(DC, 128, E).transpose(1, 0, 2)).astype(bf)
    return w1h, w2h, ctlh


def stage_consts():
    bf = ml_dtypes.bfloat16
    idf = np.eye(128, dtype=np.float32)
    idb = np.eye(128, dtype=bf)
    msk = np.zeros((128, NBLK, E), np.float32)
    for st in range(NBLK):
        msk[st * K:(st + 1) * K, st, :] = 1.0
    return idf, idb, msk.astype(bf)


def stage_x(xs):
    """xs: (B, s_local, D) fp32 -> (xg bf16 (T, D), xT bf16 (D, T))."""
    s_local = xs.shape[1]
    bf = ml_dtypes.bfloat16
    xg_h = np.ascontiguousarray(
        xs.transpose(1, 0, 2).reshape(s_local * K, D)).astype(bf)
    xT_h = np.ascontiguousarray(xg_h.T)
    return xg_h, xT_h


_MODULE_CACHE = {}


def kernel(x, lin1, lin2, controller):
    from concourse.bass_utils import run_bass_kernel_spmd

    s_local = S // N_CORES
    key = (s_local, N_CORES)
    if key not in _MODULE_CACHE:
        _MODULE_CACHE[key] = build_module(s_local, N_CORES)
    nc = _MODULE_CACHE[key]

    w1h, w2h, ctlh = stage_weights(lin1, lin2, controller)
    idf, idb, msk = stage_consts()
    in_maps = []
    for c in range(N_CORES):
        xg_h, xT_h = stage_x(x[:, c * s_local:(c + 1) * s_local, :])
        in_maps.append({"xg": xg_h, "xT": xT_h, "w1": w1h, "w2": w2h,
                        "ctl": ctlh, "idf": idf, "idb": idb, "msk": msk})

    res = run_bass_kernel_spmd(nc, in_maps, core_ids=list(range(N_CORES)))
    out_full = np.empty((B, S, D), np.float32)
    for c in range(N_CORES):
        oc = np.asarray(res.results[c]["out"]).astype(np.float32)
        out_full[:, c * s_local:(c + 1) * s_local, :] = (
            oc.reshape(s_local, K, D).transpose(1, 0, 2))
    kernel.last_results = res
    return out_full
